# revision 6
# baseline (speedup 1.0000x reference)
"""AbstractBlast v14: shuffle1 on the PE (double transpose), shuffle2 via DMA.

The SBUF<->SBUF cross-partition DMA path caps at ~75-100 GB/s (shared
internal SDMA bus), so the y-shuffle (4.2MB) moves to the tensor engine:
  s1:   y_ps[p, t] = Vt_j^T x_j          (vt col p=c*8+m holds rank m*16+c)
  T1:   yT_ps[t', (jj,tt,r)] = transpose(y_j 128x128 blocks)
  ev2:  yTall[t', tt, j, r]  (strided DVE/ACT write)
  T2:   ypp_ps[q=(j,m), t] = transpose(yTall[:, tt, :, c*8:(c+1)*8])
  ev3:  ypall[:, c, :]
  s2:   z_c[f=(m',o), t] = W2_c ypall[:, c, :]   (w2 rows q=j*8+m)
  sh2:  zall[c::16] <- z_c (DMA, 3-queue round robin); rank(P) = P
  s3:   out_o = U_o^T zall[:, o, :] + bias
PSUM pools: ypsum(2 f32) + misc(4, 1-bank bf16/f32 shared) + opsum(2) = 8.
"""

import ml_dtypes
import numpy as np

import concourse.bass as bass
import concourse.mybir as mybir
from concourse.bass_utils import run_bass_kernel_spmd
from concourse.tile import TileContext

F32 = mybir.dt.float32
BF16 = mybir.dt.bfloat16

B, T, D = 8, 1024, 4096
CT = 512
NCH = T // CT  # 2
JG = 4

_CACHE = {}


def _split_multi_waits(nc):
    n_split = 0
    for fn in nc.m.functions:
        for bb in fn.blocks:
            new_insts = []
            for inst in bb.instructions:
                si = inst.sync_info
                if si is not None and si.on_wait and len(si.on_wait) > 1:
                    waits = list(si.on_wait)
                    for w in waits[:-1]:
                        nop = mybir.InstNoOp(
                            name=f"{inst.name}-wsplit-{n_split}",
                            ins=[],
                            outs=[],
                            engine=inst.engine,
                            sync_info=mybir.SyncInfo(on_wait=[w], on_update=[]),
                        )
                        n_split += 1
                        new_insts.append(nop)
                    inst.sync_info = mybir.SyncInfo(
                        on_wait=[waits[-1]], on_update=list(si.on_update)
                    )
                new_insts.append(inst)
            bb.instructions = new_insts
    return n_split


def _build_kernel():
    nc = bass.Bass(trn_type="TRN2")
    xt_h = nc.dram_tensor("xt", [NCH, 4, 128, JG, 2, CT], BF16, kind="ExternalInput")
    vt_h = nc.dram_tensor("vt_w", [128, 16, 2, 128], BF16, kind="ExternalInput")
    w2_h = nc.dram_tensor("w2_w", [128, 16, 128], BF16, kind="ExternalInput")
    u_h = nc.dram_tensor("u_w", [128, 16, 256], BF16, kind="ExternalInput")
    b_h = nc.dram_tensor("bias_w", [128, 32], F32, kind="ExternalInput")
    id_h = nc.dram_tensor("id_w", [128, 128], BF16, kind="ExternalInput")
    ot_h = nc.dram_tensor("ot", [NCH, 8, 128, 2, 2, CT], BF16, kind="ExternalOutput")
    add = mybir.AluOpType.add
    ident = mybir.ActivationFunctionType.Identity

    with TileContext(nc) as tc:
        with (
            tc.tile_pool(name="wpool", bufs=1) as wpool,
            tc.tile_pool(name="xpool", bufs=5) as xpool,
            tc.tile_pool(name="yjpool", bufs=6) as yjpool,
            tc.tile_pool(name="ytpool", bufs=2) as ytpool,
            tc.tile_pool(name="yppool", bufs=2) as yppool,
            tc.tile_pool(name="zcpool", bufs=6) as zcpool,
            tc.tile_pool(name="zapool", bufs=2) as zapool,
            tc.tile_pool(name="opool", bufs=6) as opool,
            tc.tile_pool(name="ypsum", bufs=2, space="PSUM") as ypsum,
            tc.tile_pool(name="miscp", bufs=6, space="PSUM") as miscp,
        ):
            # ---- weights ----
            vt_t = wpool.tile([128, 16, 2, 128], BF16)
            nc.scalar.dma_start(out=vt_t, in_=vt_h[:, :, :, :])
            id_t = wpool.tile([128, 128], BF16)
            nc.scalar.dma_start(out=id_t, in_=id_h[:, :])
            w2_t = wpool.tile([128, 16, 128], BF16)
            nc.scalar.dma_start(out=w2_t, in_=w2_h[:, :, :])
            u_t = wpool.tile([128, 16, 256], BF16)
            bias_t = wpool.tile([128, 32], F32)

            # ---- x prefetch ----
            xt = {}
            for ch in range(NCH):
                for g in range(4):
                    xt[(ch, g)] = xpool.tile(
                        [128, JG, 2, CT], BF16, tag="xt", name=f"xt{ch}{g}"
                    )
            nc.sync.dma_start(out=xt[(0, 1)], in_=xt_h[0, 1])
            nc.scalar.dma_start(out=xt[(1, 0)], in_=xt_h[1, 0])
            nc.scalar.dma_start(out=u_t, in_=u_h[:, :, :])
            nc.scalar.dma_start(out=bias_t, in_=b_h[:, :])
            for ch, g in [(0, 0), (0, 2), (0, 3), (1, 1), (1, 2), (1, 3)]:
                nc.gpsimd.dma_start(out=xt[(ch, g)], in_=xt_h[ch, g])

            # yTall[ch][t', tt, c, j, m]
            yTall = [
                ytpool.tile([128, 4, 16, 16, 8], BF16, tag="ytall", name=f"ytall{i}")
                for i in range(NCH)
            ]
            ypall = [
                yppool.tile([128, 16, CT], BF16, tag="ypall", name=f"ypall{i}")
                for i in range(NCH)
            ]
            zall = [
                zapool.tile([128, 16, CT], BF16, tag="zall", name=f"zall{i}")
                for i in range(NCH)
            ]
            o_sb = {}
            yj = {}

            nev = [0]
            ndq = [0]

            def evac(out, in_):
                nev[0] += 1
                if nev[0] % 3 != 0:
                    nc.vector.tensor_copy(out, in_)
                else:
                    nc.scalar.copy(out, in_)

            def s1(ch, j):
                y_ps = ypsum.tile([128, CT], F32, tag="yps")
                for k in range(2):
                    nc.tensor.matmul(
                        y_ps,
                        vt_t[:, j, k, :],
                        xt[(ch, j // JG)][:, j % JG, k, :],
                        start=(k == 0),
                        stop=(k == 1),
                    )
                y_j = yjpool.tile([128, CT], BF16, tag="yj")
                yj[(ch, j)] = y_j
                evac(y_j, y_ps)

            def t1(ch, j0):
                # transpose a j-pair into one psum bank: cols (jj, tt, r=(c,m))
                yT = miscp.tile([128, 1024], BF16, tag="mp", name=f"yT{ch}{j0}")
                for jj in range(2):
                    y_j = yj[(ch, j0 + jj)]
                    for tt in range(4):
                        nc.tensor.transpose(
                            yT[:, jj * 512 + tt * 128 : jj * 512 + (tt + 1) * 128],
                            y_j[:, tt * 128 : (tt + 1) * 128],
                            id_t,
                        )
                # evac -> yTall[:, tt, c, j0+jj, m], src iter (jj, tt, c, m)
                dst = yTall[ch][:, :, :, j0 : j0 + 2, :].rearrange(
                    "p tt c j m -> p j tt c m"
                )
                evac(dst, yT)

            def t2(ch, c0):
                # transpose 2 colors into one psum bank -> ypall[:, c0:c0+2, :]
                yp = miscp.tile([128, 1024], BF16, tag="mp", name=f"yp{ch}{c0}")
                for cc in range(2):
                    c = c0 + cc
                    for tt in range(4):
                        nc.tensor.transpose(
                            yp[:, cc * 512 + tt * 128 : cc * 512 + (tt + 1) * 128],
                            yTall[ch][:, tt, c, :, :],
                            id_t,
                        )
                evac(ypall[ch][:, c0 : c0 + 2, :], yp)

            def s2(ch, c):
                z_ps = miscp.tile([128, CT], F32, tag="mp", name=f"zps{ch}{c}")
                nc.tensor.matmul(
                    z_ps, w2_t[:, c, :], ypall[ch][:, c, :], start=True, stop=True
                )
                z_c = zcpool.tile([128, CT], BF16, tag="zc")
                evac(z_c, z_ps)
                q = [nc.sync, nc.gpsimd][ndq[0] % 2]
                ndq[0] += 1
                q.dma_start(out=zall[ch][c::16, :, :], in_=z_c)

            def s3(ch, o):
                og, oo = o // 2, o % 2
                if (ch, og) not in o_sb:
                    o_sb[(ch, og)] = opool.tile(
                        [128, 2, 2, CT], BF16, tag="osb", name=f"osb{ch}{og}"
                    )
                ot = o_sb[(ch, og)]
                for h in range(2):
                    o_ps = miscp.tile([128, CT], F32, tag="mp", name=f"ops{ch}{o}{h}")
                    nc.tensor.matmul(
                        o_ps,
                        u_t[:, o, h * 128 : (h + 1) * 128],
                        zall[ch][:, o, :],
                        start=True,
                        stop=True,
                    )
                    if h == 0:
                        nc.scalar.activation(
                            ot[:, oo, 0, :],
                            o_ps,
                            ident,
                            bias=bias_t[:, 2 * o : 2 * o + 1],
                            scale=1.0,
                        )
                    else:
                        nc.vector.tensor_scalar(
                            ot[:, oo, 1, :],
                            o_ps,
                            bias_t[:, 2 * o + 1 : 2 * o + 2],
                            None,
                            add,
                        )
                if oo == 1:
                    if ch == 1 and og >= 4:
                        q = nc.sync if og % 2 == 0 else nc.gpsimd
                    else:
                        q = nc.scalar
                    q.dma_start(out=ot_h[ch, og], in_=ot)

            # ---- pipeline emission ----
            # P1: s1(c0) with T1 lagging 2 js
            for j in range(16):
                s1(0, j)
                if j >= 3 and j % 2 == 1:
                    t1(0, j - 3)
            t1(0, 14)
            # P2: s1(c1, j0..5) + T1(c1) lag
            for j in range(6):
                s1(1, j)
                if j >= 3 and j % 2 == 1:
                    t1(1, j - 3)
            # P3: T2/s2(c0) interleaved, 1-pair lookahead
            t2(0, 0)
            t2(0, 2)
            for c0 in range(0, 16, 2):
                if c0 + 4 < 16:
                    t2(0, c0 + 4)
                s2(0, c0)
                s2(0, c0 + 1)
            # P4: s1(c1) rest + T1(c1)
            for j in range(6, 16):
                s1(1, j)
                if j % 2 == 1:
                    t1(1, j - 3)
            t1(1, 14)
            # P5: T2/s2(c1) first half
            t2(1, 0)
            t2(1, 2)
            for c0 in range(0, 8, 2):
                t2(1, c0 + 4)
                s2(1, c0)
                s2(1, c0 + 1)
            # P6: s3(c0, o0..1)
            for o in range(2):
                s3(0, o)
            # P7: T2/s2(c1) rest
            for c0 in range(8, 16, 2):
                if c0 + 4 < 16:
                    t2(1, c0 + 4)
                s2(1, c0)
                s2(1, c0 + 1)
            # P8: s3(c0, o2..15) covers the sh2(c1) drain
            for o in range(2, 16):
                s3(0, o)
            # P9: s3(c1)
            for o in range(16):
                s3(1, o)

    _split_multi_waits(nc)
    return nc


def _prep_weights(S, U, Vt, bias):
    bf = ml_dtypes.bfloat16
    # vt col p=c*8+m holds rank m*16+c: R(p) = (p%8)*16 + p//8
    RP = np.array([(p % 8) * 16 + p // 8 for p in range(128)])
    vt_w = np.ascontiguousarray(
        Vt[:, :, RP].reshape(16, 2, 128, 128).transpose(2, 0, 1, 3).astype(bf)
    )
    # w2_w[q=j*8+m, c, f=m'*16+o] = S[o, j, 16m'+c] * [m==m']
    S4 = S.reshape(16, 16, 8, 16)  # [o, j, m', c]
    w2 = np.zeros((16, 8, 16, 8, 16), dtype=np.float32)  # [j, m, c, m', o]
    for m in range(8):
        w2[:, m, :, m, :] = S4[:, :, m, :].transpose(1, 2, 0)
    w2_w = np.ascontiguousarray(w2.reshape(128, 16, 128).astype(bf))
    # zall[P] carries rank P (P = 16m'+c)
    u_w = np.ascontiguousarray(U.transpose(1, 0, 2).astype(bf))
    bias_w = np.ascontiguousarray(
        bias.reshape(16, 2, 128).transpose(2, 0, 1).reshape(128, 32)
    )
    id_w = np.eye(128, dtype=bf)
    return vt_w, w2_w, u_w, bias_w, id_w


def _prep_x(xb):
    bf = ml_dtypes.bfloat16
    xr = xb.T.reshape(4, JG, 2, 128, NCH, CT)  # [g, jj, k, s, ch, tt]
    return np.ascontiguousarray(xr.transpose(4, 0, 3, 1, 2, 5).astype(bf))


def _unpack_out(o):
    return np.ascontiguousarray(
        o.transpose(0, 5, 1, 3, 4, 2).reshape(T, D).astype(np.float32)
    )


def kernel(x, S, U, Vt, bias):
    x = np.asarray(x, dtype=np.float32)
    S = np.asarray(S, dtype=np.float32)
    U = np.asarray(U, dtype=np.float32)
    Vt = np.asarray(Vt, dtype=np.float32)
    bias = np.asarray(bias, dtype=np.float32)

    vt_w, w2_w, u_w, bias_w, id_w = _prep_weights(S, U, Vt, bias)

    if "nc" not in _CACHE:
        _CACHE["nc"] = _build_kernel()
    nc = _CACHE["nc"]

    in_maps = []
    for b in range(B):
        in_maps.append(
            {
                "xt": _prep_x(x[b]),
                "vt_w": vt_w,
                "w2_w": w2_w,
                "u_w": u_w,
                "bias_w": bias_w,
                "id_w": id_w,
            }
        )

    res = run_bass_kernel_spmd(nc, in_maps, core_ids=list(range(B)))

    out = np.empty((B, T, D), dtype=np.float32)
    for b in range(B):
        out[b] = _unpack_out(res.results[b]["ot"])
    return out
